# revision 4
# baseline (speedup 1.0000x reference)
"""Trainium2 Bass kernel for nn_Attention_39384850104955 (single-launch).

Dense multi-head attention (B=2, S=2048, D=1024, H=16, dh=64) with a
materialized [B,H,S,S] score tensor plus additive bias, eager softmax,
and in/out projections.

Sharding: head-parallel across 8 NeuronCores; core c owns heads
{2c, 2c+1} for BOTH batches, so each bias head is read exactly once
across the fleet. Per core:
 - QKV projections for its 2 heads (fp16), with v PE-transposed into
   [sk, dh] tiles that carry a 64-wide ones block, so attn@v emits the
   softmax row-sums replicated across 64 PSUM partitions (no gpsimd
   broadcast anywhere, keeping GpSimd free for collectives).
 - scoresT = k^T q in PSUM (no bias injection); exp on ScalarE; the
   bias enters via the softmax identity exp(s+b) = exp(s)*exp(b) with
   exp(bias) precomputed on host and multiplied in on VectorE (fp16,
   2x rate). attn@v accumulates incrementally per tile pair, so only a
   small ring of exp tiles lives in SBUF.
 - The per-(batch, sq-block) context slices [128 dims, 512 rows] are
   exchanged with two 1 MB AllToAlls (fp16, bypass) so each core ends
   up with the full hidden dim for its 512 output rows, then does the
   full output projection locally (fp32 PSUM accumulate) - no fp16
   partial-sum ReduceScatter, no 16 MB DRAM round-trip. A2A#1 covers
   sq-blocks 0-1 and overlaps blocks 2-3 compute; its output projection
   is emitted after sq-block 2 so the PE never stalls on the fabric.
Output assembled on host by concatenating the 8 per-core row shards.
"""

import sys

sys.path.insert(0, "/opt/trn_rl_repo")

import numpy as np

import concourse.bacc as bacc
import concourse.mybir as mybir
import concourse.tile as tile
from concourse.bass_utils import run_bass_kernel_spmd

f32 = mybir.dt.float32
f16 = mybir.dt.float16  # fp16: same PE/DVE speed as bf16, 8x the precision
f32r = mybir.dt.float32r

P = 128
B, S, D, H, DH = 2, 2048, 1024, 16, 64
NCORE = 8
NT = S // P           # 16 sk tiles per batch
SBLK = 512            # projection seq block
SQB = 512             # attention sq block
NQB = S // SQB        # 4 sq blocks
SCALE = 1.0 / 8.0     # 1/sqrt(dh)

Exp = mybir.ActivationFunctionType.Exp
Mult = mybir.AluOpType.mult

_CACHE = {}


def _emit_oproj(nc, ctx_pool, pf_pool, pt_pool, woT_s, rcv, fin_ab, skip):
    """DMA rcv -> SBUF ctx, full output projection, store to fin[ab]."""
    ctx = ctx_pool.tile([P, 8, SQB], f16, tag="ctx", name="ctx")
    if "a2a" not in skip:
        nc.sync.dma_start(ctx[:], rcv.rearrange("s p n -> p s n"))
    else:
        nc.vector.memset(ctx[0:1, 0, 0:1], 0.0)
    nsrc = 1 if "oproj" in skip else 8
    for rt in range(4):
        for nh in range(2):
            pf = pf_pool.tile([P, 512], f32, tag="pf", name="pf")
            for s in range(nsrc):
                nc.tensor.matmul(pf[:],
                                 ctx[:, s, rt * P:(rt + 1) * P],
                                 woT_s[:, s, nh * 512:(nh + 1) * 512],
                                 start=(s == 0), stop=(s == nsrc - 1))
            pt = pt_pool.tile([P, 512], f16, tag="pt", name="pt")
            nc.vector.tensor_copy(pt[:], pf[:])
            nc.sync.dma_start(
                fin_ab[rt * P:(rt + 1) * P, nh * 512:(nh + 1) * 512], pt[:])


def _emit_body(nc, tc, ident16, w_s, woT_s, xT, expb, snds, rcvs, fin,
               qk_pool, skip=()):
    qT_s = qk_pool.tile([P, B * S], f16, tag="qT", name="qT")  # pre-scaled
    kT_s = qk_pool.tile([P, B * S], f16, tag="kT", name="kT")
    # v tiles [sk, 2*128]: per head 64 v-dims then 64 ones columns, so
    # attn@v yields rows 0:64 = context, rows 64:128 = replicated row-sum
    v_s = qk_pool.tile([P, 2 * NT, 256], f16, tag="v", name="v")
    nc.vector.memset(v_s[:, :, 64:128], 1.0)
    nc.vector.memset(v_s[:, :, 192:256], 1.0)

    # ---------------- QKV projections ----------------
    with tc.tile_pool(name="xload", bufs=2) as xload, \
         tc.tile_pool(name="vstage", bufs=2) as vstage, \
         tc.tile_pool(name="pp", bufs=2, space="PSUM") as pp:
        for sb in range(8):
            s0 = sb * SBLK
            xt = xload.tile([P, 8, SBLK], f16, tag="xt", name="xt")
            if "xdma" not in skip:
                nc.sync.dma_start(
                    xt[:], xT[:, s0:s0 + SBLK].rearrange("(c p) n -> p c n", p=P))
            else:
                nc.vector.memset(xt[0:1, 0, 0:1], 0.0)
            pq = pp.tile([P, SBLK], f32, tag="pq", name="pq")
            pk = pp.tile([P, SBLK], f32, tag="pk", name="pk")
            pv = pp.tile([P, SBLK], f32, tag="pv", name="pv")
            nch = 1 if "proj" in skip else 8
            for c in range(nch):
                st, sp = (c == 0), (c == nch - 1)
                nc.tensor.matmul(pq[:], w_s[:, 0, c, :], xt[:, c, :],
                                 start=st, stop=sp)
                nc.tensor.matmul(pk[:], w_s[:, 1, c, :], xt[:, c, :],
                                 start=st, stop=sp)
                nc.tensor.matmul(pv[:], w_s[:, 2, c, :], xt[:, c, :],
                                 start=st, stop=sp)
            nc.vector.tensor_scalar_mul(qT_s[:, s0:s0 + SBLK], pq[:], SCALE)
            nc.vector.tensor_copy(kT_s[:, s0:s0 + SBLK], pk[:])
            # v: transpose [dout, s] -> [s, dout] tiles, store fp16
            vst = vstage.tile([P, SBLK], f16, tag="vst", name="vst")
            nc.vector.tensor_copy(vst[:], pv[:])
            for a in range(4):
                pvt = pp.tile([P, P], f16, tag="pvt", name="pvt")
                nc.tensor.matmul(pvt[:],
                                 vst[:, a * P:(a + 1) * P], ident16[:],
                                 is_transpose=True, start=True, stop=True)
                g = sb * 4 + a  # global sk tile 0..31 (= bb*16 + t)
                nc.vector.tensor_copy(v_s[:, g, 0:64], pvt[:, 0:64])
                nc.vector.tensor_copy(v_s[:, g, 128:192], pvt[:, 64:128])

    # ---------------- attention + A2A + output projection ----------------
    hsl = [slice(0, 64), slice(64, 128)]
    with tc.tile_pool(name="slab", bufs=4) as slab_pool, \
         tc.tile_pool(name="expp", bufs=3) as exp_pool, \
         tc.tile_pool(name="nrm", bufs=2) as nrm_pool, \
         tc.tile_pool(name="ost", bufs=2) as ost_pool, \
         tc.tile_pool(name="ctxp", bufs=2) as ctx_pool, \
         tc.tile_pool(name="ptp", bufs=3) as pt_pool, \
         tc.tile_pool(name="sc", bufs=2, space="PSUM") as sc_pool, \
         tc.tile_pool(name="av", bufs=2, space="PSUM") as av_pool, \
         tc.tile_pool(name="pf", bufs=2, space="PSUM") as pf_pool:
        for sqb in range(NQB):
            sq0 = sqb * SQB
            slabs = {}
            for h in range(2):
                slb = slab_pool.tile([P, NT, SQB], f16, tag="slab",
                                     name=f"slab_{h}")
                if "bdma" not in skip:
                    nc.sync.dma_start(
                        slb[:],
                        expb[h, :, sq0:sq0 + SQB]
                        .rearrange("(t p) n -> p t n", p=P))
                else:
                    nc.vector.memset(slb[0:1, 0, 0:1], 0.0)
                slabs[h] = slb
            for bb in range(2):
                oc = ost_pool.tile([P, SQB], f16, tag="oc", name=f"oc{bb}")
                for h in range(2):
                    pa = av_pool.tile([P, SQB], f32, tag="av", name="av")
                    ng = 1 if "attnv" in skip else 8
                    for g in range(ng):
                        psg = sc_pool.tile([P, 2 * SQB], f32, tag="sc",
                                           name="sc")
                        for j in range(2):
                            t = 2 * g + j
                            nc.tensor.matmul(
                                psg[:, j * SQB:(j + 1) * SQB],
                                kT_s[hsl[h], bb * S + t * P:bb * S + (t + 1) * P],
                                qT_s[hsl[h], bb * S + sq0:bb * S + sq0 + SQB],
                                start=True, stop=True)
                        expt = exp_pool.tile([P, 2, SQB], f16, tag="exp",
                                             name="exp")
                        if "exp" in skip:
                            nc.scalar.activation(expt[:, 0, 0:P],
                                                 psg[:, 0:P], Exp)
                        else:
                            nc.scalar.activation(expt[:], psg[:], Exp)
                        if "mult" not in skip:
                            nc.vector.tensor_tensor(
                                expt[:], expt[:],
                                slabs[h][:, 2 * g:2 * g + 2, :], Mult)
                        for j in range(2):
                            t = 2 * g + j
                            nc.tensor.matmul(
                                pa[:], v_s[:, bb * NT + t, h * P:(h + 1) * P],
                                expt[:, j, :],
                                start=(g == 0 and j == 0),
                                stop=(g == ng - 1 and j == 1))
                    rbc = nrm_pool.tile([64, SQB], f32, tag="rbc", name="rbc")
                    nc.vector.reciprocal(rbc[:], pa[64:128, :])
                    nc.vector.tensor_tensor(oc[hsl[h], :], pa[0:64, :],
                                            rbc[:], Mult)
                # ship this (bb, sqb) context chunk to its owner core
                ab = sqb // 2
                d = bb * 4 + sqb
                nc.sync.dma_start(snds[ab][d], oc[:])
            if sqb == 1 or sqb == 3:
                ab = sqb // 2
                if "a2a" not in skip:
                    nc.gpsimd.collective_compute(
                        "AllToAll", mybir.AluOpType.bypass,
                        replica_groups=[list(range(NCORE))],
                        ins=[snds[ab]], outs=[rcvs[ab]])
            if sqb == 2 or sqb == 3:
                # oproj for A2A#1 lands after sqb2 compute (fabric overlap);
                # oproj for A2A#2 is the tail
                ab = sqb - 2
                _emit_oproj(nc, ctx_pool, pf_pool, pt_pool, woT_s,
                            rcvs[ab], fin[ab], skip)


def build_full(repeat=1, skip=()):
    nc = bacc.Bacc("TRN2", target_bir_lowering=False, debug=False,
                   num_devices=NCORE)
    xT = nc.dram_tensor("xT", [D, B * S], f16, kind="ExternalInput").ap()
    wT = nc.dram_tensor("wT", [3, D, P], f16, kind="ExternalInput").ap()
    expb = nc.dram_tensor("expb", [2, S, S], f16, kind="ExternalInput").ap()
    identr = nc.dram_tensor("identr", [P, P], f32r, kind="ExternalInput").ap()
    woT = nc.dram_tensor("woT", [D, D], f16, kind="ExternalInput").ap()
    fin = nc.dram_tensor("fin", [2, SQB, D], f16, kind="ExternalOutput").ap()
    snds = [nc.dram_tensor(f"snd{a}", [NCORE, P, SQB], f16).ap()
            for a in range(2)]
    rcvs = [nc.dram_tensor(f"rcv{a}", [NCORE, P, SQB], f16).ap()
            for a in range(2)]

    with tile.TileContext(nc) as tc:
        with tc.tile_pool(name="const", bufs=1) as const_pool, \
             tc.tile_pool(name="qk", bufs=1) as qk_pool:
            ident_s = const_pool.tile([P, P], f32r, tag="ident", name="ident")
            nc.sync.dma_start(ident_s[:], identr)
            ident16 = const_pool.tile([P, P], f16, tag="ident16",
                                      name="ident16")
            nc.vector.tensor_copy(ident16[:], ident_s[:].bitcast(f32))
            w_s = const_pool.tile([P, 3, 8, P], f16, tag="w", name="w")
            nc.sync.dma_start(w_s[:], wT.rearrange("w (c p) m -> p w c m", p=P))
            woT_s = const_pool.tile([P, 8, D], f16, tag="woT", name="woT")
            nc.sync.dma_start(woT_s[:], woT.rearrange("(s p) n -> p s n", p=P))
            for _rep in range(repeat):
                _emit_body(nc, tc, ident16, w_s, woT_s, xT, expb, snds, rcvs,
                           fin, qk_pool, skip=skip)

    nc.compile()
    return nc


def _get(name, builder):
    if name not in _CACHE:
        _CACHE[name] = builder()
    return _CACHE[name]


def make_in_maps(hidden_states, bias, Wq, Wk, Wv, Wo):
    xT = np.ascontiguousarray(
        hidden_states.reshape(B * S, D).T).astype(np.float16)
    # exp(bias) transposed per head to [H, sk, sq], fp16 (host prep)
    expb_all = np.exp(bias[0].transpose(0, 2, 1)).astype(np.float16)
    ident = np.eye(P, dtype=np.float32)
    woT = np.ascontiguousarray(Wo.T).astype(np.float16)
    in_maps = []
    for c in range(NCORE):
        r0 = c * 2 * DH
        wTc = np.stack([np.ascontiguousarray(W[r0:r0 + 2 * DH, :].T)
                        .astype(np.float16) for W in (Wq, Wk, Wv)])
        in_maps.append({
            "xT": xT,
            "wT": wTc,
            "expb": np.ascontiguousarray(expb_all[2 * c:2 * c + 2]),
            "identr": ident,
            "woT": woT,
        })
    return in_maps


def assemble(results):
    out = np.empty((B * S, D), dtype=np.float32)
    for c in range(NCORE):
        bb, sqb = c // 4, c % 4
        ab = 0 if sqb < 2 else 1
        finc = np.asarray(results[c]["fin"], dtype=np.float32)
        r0 = bb * S + sqb * SQB
        out[r0:r0 + SQB] = finc[ab]
    return out.reshape(B, S, D)


def kernel(hidden_states, bias, Wq, Wk, Wv, Wo):
    hidden_states = np.ascontiguousarray(hidden_states, dtype=np.float32)
    bias = np.ascontiguousarray(bias, dtype=np.float32)
    Wq = np.ascontiguousarray(Wq, dtype=np.float32)
    Wk = np.ascontiguousarray(Wk, dtype=np.float32)
    Wv = np.ascontiguousarray(Wv, dtype=np.float32)
    Wo = np.ascontiguousarray(Wo, dtype=np.float32)

    nc = _get("full", build_full)
    in_maps = make_in_maps(hidden_states, bias, Wq, Wk, Wv, Wo)
    res = run_bass_kernel_spmd(nc, in_maps, list(range(NCORE))).results
    return assemble(res)


# revision 13
# speedup vs baseline: 1.0090x; 1.0090x over previous
"""Trainium2 Bass kernel for nn_Attention_39384850104955 (single-launch).

Dense multi-head attention (B=2, S=2048, D=1024, H=16, dh=64) with a
materialized [B,H,S,S] score tensor plus additive bias, eager softmax,
and in/out projections.

Sharding: head-parallel across 8 NeuronCores; core c owns heads
{2c, 2c+1} for BOTH batches, so each bias head is read exactly once
across the fleet. Per core:
 - QKV projections for its 2 heads (fp16), with v PE-transposed into
   [sk, dh] tiles that carry a 64-wide ones block, so attn@v emits the
   softmax row-sums replicated across 64 PSUM partitions (no gpsimd
   broadcast anywhere, keeping GpSimd free for collectives).
 - scoresT = k^T q in PSUM (no bias injection); exp on ScalarE; the
   bias enters via the softmax identity exp(s+b) = exp(s)*exp(b) with
   exp(bias) precomputed on host and multiplied in on VectorE (fp16,
   2x rate). attn@v accumulates incrementally per tile pair, so only a
   small ring of exp tiles lives in SBUF.
 - The per-(batch, sq-block) context slices [128 dims, 512 rows] are
   exchanged with two 1 MB AllToAlls (fp16, bypass) so each core ends
   up with the full hidden dim for its 512 output rows, then does the
   full output projection locally (fp32 PSUM accumulate) - no fp16
   partial-sum ReduceScatter, no 16 MB DRAM round-trip. A2A#1 covers
   sq-blocks 0-1 and overlaps blocks 2-3 compute; its output projection
   is emitted after sq-block 2 so the PE never stalls on the fabric.
Output assembled on host by concatenating the 8 per-core row shards.
"""

import sys

sys.path.insert(0, "/opt/trn_rl_repo")

import numpy as np

import concourse.bacc as bacc
import concourse.mybir as mybir
import concourse.tile as tile
from concourse.bass_utils import run_bass_kernel_spmd

f32 = mybir.dt.float32
f16 = mybir.dt.float16  # fp16: same PE/DVE speed as bf16, 8x the precision
f32r = mybir.dt.float32r

P = 128
B, S, D, H, DH = 2, 2048, 1024, 16, 64
NCORE = 8
NT = S // P           # 16 sk tiles per batch
SBLK = 512            # projection seq block
SQB = 512             # attention sq block
NQB = S // SQB        # 4 sq blocks
SCALE = 1.0 / 8.0     # 1/sqrt(dh)

Exp = mybir.ActivationFunctionType.Exp
Mult = mybir.AluOpType.mult

_CACHE = {}


def _emit_oproj(nc, ctx_pool, pf_pool, pt_pool, woT_s, rcv, fin, ab, msel_s,
                ptA_s, skip):
    """DMA rcv -> SBUF ctx, full output projection.

    Cores cannot branch on which A2A carried their valid context, so both
    projections run everywhere and a per-core (mA, mB) in {(1,0),(0,1)}
    mask input selects: ab=0 stages pf*mA in SBUF; ab=1 computes
    pf*mB + staged and stores the single blended fin (1 MB output)."""
    ctx = ctx_pool.tile([P, 8, SQB], f16, tag="ctx", name="ctx")
    if "a2a" not in skip:
        nc.sync.dma_start(ctx[:], rcv.rearrange("s p n -> p s n"))
    else:
        nc.vector.memset(ctx[0:1, 0, 0:1], 0.0)
    nsrc = 1 if "oproj" in skip else 8
    for rt in range(4):
        for nh in range(2):
            pf = pf_pool.tile([P, 512], f32, tag="pf", name="pf")
            for s in range(nsrc):
                nc.tensor.matmul(pf[:],
                                 ctx[:, s, rt * P:(rt + 1) * P],
                                 woT_s[:, s, nh * 512:(nh + 1) * 512],
                                 start=(s == 0), stop=(s == nsrc - 1))
            i = rt * 2 + nh
            if ab == 0:
                nc.vector.tensor_scalar_mul(ptA_s[:, i, :], pf[:],
                                            msel_s[:, 0:1])
            else:
                pt = pt_pool.tile([P, 512], f16, tag="pt", name="pt")
                nc.vector.scalar_tensor_tensor(
                    pt[:], pf[:], msel_s[:, 1:2], ptA_s[:, i, :],
                    mybir.AluOpType.mult, mybir.AluOpType.add)
                nc.sync.dma_start(
                    fin[rt * P:(rt + 1) * P, nh * 512:(nh + 1) * 512], pt[:])


def _emit_body(nc, tc, ident16, w_s, woT_s, msel_s, xT, expb, snds, rcvs, fin,
               qk_pool, skip=()):
    qT_s = qk_pool.tile([P, B * S], f16, tag="qT", name="qT")  # pre-scaled
    kT_s = qk_pool.tile([P, B * S], f16, tag="kT", name="kT")
    ptA_s = qk_pool.tile([P, 8, 512], f16, tag="ptA", name="ptA")
    # v tiles [sk, 2*128]: per head 64 v-dims then 64 ones columns, so
    # attn@v yields rows 0:64 = context, rows 64:128 = replicated row-sum
    v_s = qk_pool.tile([P, 2 * NT, 256], f16, tag="v", name="v")
    nc.vector.memset(v_s[:, :, 64:128], 1.0)
    nc.vector.memset(v_s[:, :, 192:256], 1.0)

    # ---------------- QKV projections ----------------
    with tc.tile_pool(name="xload", bufs=2) as xload, \
         tc.tile_pool(name="vstage", bufs=2) as vstage, \
         tc.tile_pool(name="pp", bufs=2, space="PSUM") as pp:
        for sb in range(8):
            s0 = sb * SBLK
            xt = xload.tile([P, 8, SBLK], f16, tag="xt", name="xt")
            if "xdma" not in skip:
                nc.sync.dma_start(
                    xt[:], xT[:, s0:s0 + SBLK].rearrange("(c p) n -> p c n", p=P))
            else:
                nc.vector.memset(xt[0:1, 0, 0:1], 0.0)
            pq = pp.tile([P, SBLK], f32, tag="pq", name="pq")
            pk = pp.tile([P, SBLK], f32, tag="pk", name="pk")
            pv = pp.tile([P, SBLK], f32, tag="pv", name="pv")
            nch = 1 if "proj" in skip else 8
            for c in range(nch):
                st, sp = (c == 0), (c == nch - 1)
                nc.tensor.matmul(pq[:], w_s[:, 0, c, :], xt[:, c, :],
                                 start=st, stop=sp)
                nc.tensor.matmul(pk[:], w_s[:, 1, c, :], xt[:, c, :],
                                 start=st, stop=sp)
                nc.tensor.matmul(pv[:], w_s[:, 2, c, :], xt[:, c, :],
                                 start=st, stop=sp)
            nc.vector.tensor_scalar_mul(qT_s[:, s0:s0 + SBLK], pq[:], SCALE)
            nc.vector.tensor_copy(kT_s[:, s0:s0 + SBLK], pk[:])
            # v: transpose [dout, s] -> [s, dout] tiles, store fp16
            vst = vstage.tile([P, SBLK], f16, tag="vst", name="vst")
            nc.vector.tensor_copy(vst[:], pv[:])
            for a in range(4):
                pvt = pp.tile([P, P], f16, tag="pvt", name="pvt")
                nc.tensor.matmul(pvt[:],
                                 vst[:, a * P:(a + 1) * P], ident16[:],
                                 is_transpose=True, start=True, stop=True)
                g = sb * 4 + a  # global sk tile 0..31 (= bb*16 + t)
                nc.vector.tensor_copy(v_s[:, g, 0:64], pvt[:, 0:64])
                nc.vector.tensor_copy(v_s[:, g, 128:192], pvt[:, 64:128])

    # ---------------- attention + A2A + output projection ----------------
    hsl = [slice(0, 64), slice(64, 128)]
    with tc.tile_pool(name="slab", bufs=4) as slab_pool, \
         tc.tile_pool(name="expp", bufs=3) as exp_pool, \
         tc.tile_pool(name="nrm", bufs=2) as nrm_pool, \
         tc.tile_pool(name="ost", bufs=2) as ost_pool, \
         tc.tile_pool(name="ctxp", bufs=2) as ctx_pool, \
         tc.tile_pool(name="ptp", bufs=3) as pt_pool, \
         tc.tile_pool(name="sc", bufs=2, space="PSUM") as sc_pool, \
         tc.tile_pool(name="av", bufs=2, space="PSUM") as av_pool, \
         tc.tile_pool(name="pf", bufs=2, space="PSUM") as pf_pool:
        for sqb in range(NQB):
            sq0 = sqb * SQB
            slabs = {}
            for h in range(2):
                slb = slab_pool.tile([P, NT, SQB], f16, tag="slab",
                                     name=f"slab_{h}")
                if "bdma" not in skip:
                    nc.sync.dma_start(
                        slb[:],
                        expb[h, :, sq0:sq0 + SQB]
                        .rearrange("(t p) n -> p t n", p=P))
                else:
                    nc.vector.memset(slb[0:1, 0, 0:1], 0.0)
                slabs[h] = slb
            for bb in range(2):
                oc = ost_pool.tile([P, SQB], f16, tag="oc", name=f"oc{bb}")
                for h in range(2):
                    pa = av_pool.tile([P, SQB], f32, tag="av", name="av")
                    ng = 1 if "attnv" in skip else 8
                    for g in range(ng):
                        psg = sc_pool.tile([P, 2 * SQB], f32, tag="sc",
                                           name="sc")
                        for j in range(2):
                            t = 2 * g + j
                            nc.tensor.matmul(
                                psg[:, j * SQB:(j + 1) * SQB],
                                kT_s[hsl[h], bb * S + t * P:bb * S + (t + 1) * P],
                                qT_s[hsl[h], bb * S + sq0:bb * S + sq0 + SQB],
                                start=True, stop=True)
                        expt = exp_pool.tile([P, 2, SQB], f16, tag="exp",
                                             name="exp")
                        if "exp" in skip:
                            nc.scalar.activation(expt[:, 0, 0:P],
                                                 psg[:, 0:P], Exp)
                        else:
                            nc.scalar.activation(expt[:], psg[:], Exp)
                        if "mult" not in skip:
                            nc.vector.tensor_tensor(
                                expt[:], expt[:],
                                slabs[h][:, 2 * g:2 * g + 2, :], Mult)
                        for j in range(2):
                            t = 2 * g + j
                            nc.tensor.matmul(
                                pa[:], v_s[:, bb * NT + t, h * P:(h + 1) * P],
                                expt[:, j, :],
                                start=(g == 0 and j == 0),
                                stop=(g == ng - 1 and j == 1))
                    rbc = nrm_pool.tile([64, SQB], f32, tag="rbc", name="rbc")
                    nc.vector.reciprocal(rbc[:], pa[64:128, :])
                    nc.vector.tensor_tensor(oc[hsl[h], :], pa[0:64, :],
                                            rbc[:], Mult)
                # ship this (bb, sqb) context chunk to its owner core
                ab = sqb // 2
                d = bb * 4 + sqb
                nc.sync.dma_start(snds[ab][d], oc[:])
                # prefill the blocks this A2A never writes with finite data
                # (the unselected projection multiplies them by 0; Inf/NaN
                # from uninitialized DRAM would poison the blend as 0*Inf)
                if sqb == 0:
                    nc.sync.dma_start(snds[0][bb * 4 + 2], oc[:])
                    nc.sync.dma_start(snds[0][bb * 4 + 3], oc[:])
                elif sqb == 2:
                    nc.sync.dma_start(snds[1][bb * 4 + 0], oc[:])
                    nc.sync.dma_start(snds[1][bb * 4 + 1], oc[:])
            if sqb == 1 or sqb == 3:
                ab = sqb // 2
                if "a2a" not in skip:
                    nc.gpsimd.collective_compute(
                        "AllToAll", mybir.AluOpType.bypass,
                        replica_groups=[list(range(NCORE))],
                        ins=[snds[ab]], outs=[rcvs[ab]])
            if sqb == 2 or sqb == 3:
                # oproj for A2A#1 lands after sqb2 compute (fabric overlap);
                # oproj for A2A#2 is the tail
                ab = sqb - 2
                _emit_oproj(nc, ctx_pool, pf_pool, pt_pool, woT_s,
                            rcvs[ab], fin, ab, msel_s, ptA_s, skip)


def build_full(repeat=1, skip=()):
    nc = bacc.Bacc("TRN2", target_bir_lowering=False, debug=False,
                   num_devices=NCORE)
    xT = nc.dram_tensor("xT", [D, B * S], f16, kind="ExternalInput").ap()
    wT = nc.dram_tensor("wT", [3, D, P], f16, kind="ExternalInput").ap()
    expb = nc.dram_tensor("expb", [2, S, S], f16, kind="ExternalInput").ap()
    identr = nc.dram_tensor("identr", [P, P], f32r, kind="ExternalInput").ap()
    woT = nc.dram_tensor("woT", [D, D], f16, kind="ExternalInput").ap()
    msel = nc.dram_tensor("msel", [P, 2], f32, kind="ExternalInput").ap()
    fin = nc.dram_tensor("fin", [SQB, D], f16, kind="ExternalOutput").ap()
    snds = [nc.dram_tensor(f"snd{a}", [NCORE, P, SQB], f16).ap()
            for a in range(2)]
    rcvs = [nc.dram_tensor(f"rcv{a}", [NCORE, P, SQB], f16).ap()
            for a in range(2)]

    with tile.TileContext(nc) as tc:
        with tc.tile_pool(name="const", bufs=1) as const_pool, \
             tc.tile_pool(name="qk", bufs=1) as qk_pool:
            ident_s = const_pool.tile([P, P], f32r, tag="ident", name="ident")
            nc.sync.dma_start(ident_s[:], identr)
            ident16 = const_pool.tile([P, P], f16, tag="ident16",
                                      name="ident16")
            nc.vector.tensor_copy(ident16[:], ident_s[:].bitcast(f32))
            w_s = const_pool.tile([P, 3, 8, P], f16, tag="w", name="w")
            nc.sync.dma_start(w_s[:], wT.rearrange("w (c p) m -> p w c m", p=P))
            woT_s = const_pool.tile([P, 8, D], f16, tag="woT", name="woT")
            nc.sync.dma_start(woT_s[:], woT.rearrange("(s p) n -> p s n", p=P))
            msel_s = const_pool.tile([P, 2], f32, tag="msel", name="msel")
            nc.sync.dma_start(msel_s[:], msel)
            for _rep in range(repeat):
                _emit_body(nc, tc, ident16, w_s, woT_s, msel_s, xT, expb,
                           snds, rcvs, fin, qk_pool, skip=skip)

    nc.compile()
    return nc


def _get(name, builder):
    if name not in _CACHE:
        _CACHE[name] = builder()
    return _CACHE[name]


def make_in_maps(hidden_states, bias, Wq, Wk, Wv, Wo):
    xT = np.ascontiguousarray(
        hidden_states.reshape(B * S, D).T).astype(np.float16)
    # exp(bias) transposed per head to [H, sk, sq], fp16 (host prep)
    expb_all = np.exp(bias[0].transpose(0, 2, 1)).astype(np.float16)
    ident = np.eye(P, dtype=np.float32)
    woT = np.ascontiguousarray(Wo.T).astype(np.float16)
    in_maps = []
    for c in range(NCORE):
        r0 = c * 2 * DH
        wTc = np.stack([np.ascontiguousarray(W[r0:r0 + 2 * DH, :].T)
                        .astype(np.float16) for W in (Wq, Wk, Wv)])
        # core c owns chunk (bb=c//4, sqb=c%4); sqb<2 arrives via A2A#1
        mA = 1.0 if (c % 4) < 2 else 0.0
        msel = np.tile(np.asarray([[mA, 1.0 - mA]], np.float32), (P, 1))
        in_maps.append({
            "xT": xT,
            "wT": wTc,
            "expb": np.ascontiguousarray(expb_all[2 * c:2 * c + 2]),
            "identr": ident,
            "woT": woT,
            "msel": msel,
        })
    return in_maps


def assemble(results):
    out = np.empty((B * S, D), dtype=np.float32)
    for c in range(NCORE):
        bb, sqb = c // 4, c % 4
        finc = np.asarray(results[c]["fin"], dtype=np.float32)
        r0 = bb * S + sqb * SQB
        out[r0:r0 + SQB] = finc
    return out.reshape(B, S, D)


def kernel(hidden_states, bias, Wq, Wk, Wv, Wo):
    hidden_states = np.ascontiguousarray(hidden_states, dtype=np.float32)
    bias = np.ascontiguousarray(bias, dtype=np.float32)
    Wq = np.ascontiguousarray(Wq, dtype=np.float32)
    Wk = np.ascontiguousarray(Wk, dtype=np.float32)
    Wv = np.ascontiguousarray(Wv, dtype=np.float32)
    Wo = np.ascontiguousarray(Wo, dtype=np.float32)

    nc = _get("full", build_full)
    in_maps = make_in_maps(hidden_states, bias, Wq, Wk, Wv, Wo)
    res = run_bass_kernel_spmd(nc, in_maps, list(range(NCORE))).results
    return assemble(res)
